# revision 1
# baseline (speedup 1.0000x reference)
"""Trainium2 Bass kernel for nn_NodeBlock (GNN message passing).

Pipeline: segment_sum of edge features onto destination nodes, concat with
node features, 3-layer MLP, LayerNorm.

Sharding: nodes are range-sharded across the 8 cores (12800 nodes/core, 100
blocks of 128). On the host, edges are bucketed by destination-node block
(a shard of the edge list per core, padded per block-position to a uniform
tile count Kb across cores), so each core streams only the edge rows it
needs, contiguously. Edge values are split hi/lo into two fp16 tensors
(hi = fp16(x), lo = fp16(x - hi), ~2e-7 combined relative error) so the
PE runs at full 1-cycle/row rate instead of fp32's 4 cycles/row.

On device, per 128-node block: the segment sum is a pair of one-hot fp16
matmuls accumulated in PSUM (aggrT[f, j] = sum_e hi[e, f] * oh[e, j] +
lo[e, f] * oh[e, j], oh = (col_local == j)), followed by the f32 MLP in
feature-major layout and a PE transpose + LayerNorm.
"""

import sys

sys.path.insert(0, "/opt/trn_rl_repo")

import numpy as np

N_CORES = 8
NUM_NODES = 100000
D = 128            # node/edge feature dim
P = 128            # partitions
BLK = 128          # nodes per block
BLOCKS_PER_CORE = 100
NODES_PER_CORE = BLK * BLOCKS_PER_CORE   # 12800
TOTAL_BLOCKS = N_CORES * BLOCKS_PER_CORE  # 800
EPS = 1e-5

_nc_cache = {}
last_run_info = {}


TUNE = {"ebufs": 4, "ohbufs": 3, "sbufs": 4, "agbufs": 3, "mlpbufs": 3,
        "oh_mode": "ts", "dma_split": True, "edge_pair": False,
        "only": None}


def _build_nc(kb, loop_iters=None):
    """kb: tuple of per-block-position edge-tile counts (len 100)."""
    import contextlib
    import concourse.bacc as bacc
    import concourse.tile as tile
    import concourse.mybir as mybir
    from concourse.masks import make_identity

    dt = mybir.dt
    f32 = dt.float32
    f16 = dt.float16
    kb = list(kb)
    kmax = max(kb)
    tot_e = sum(k * 256 for k in kb)   # per-partition fp16 elems (hi+lo)
    tot_c = sum(kb)

    nc = bacc.Bacc("TRN2", target_bir_lowering=False, debug=False,
                   name="nodeblock")

    edges = nc.dram_tensor("edges", [P, tot_e], f16, kind="ExternalInput")
    colf = nc.dram_tensor("colf", [P, tot_c], f16, kind="ExternalInput")
    colf32 = nc.dram_tensor("colf32", [P, tot_c], f32, kind="ExternalInput")
    natT = nc.dram_tensor("natT", [P, NODES_PER_CORE], f32,
                          kind="ExternalInput")
    iota = nc.dram_tensor("iota", [P, kmax, 128], f16, kind="ExternalInput")
    w_in = {}
    for nm in ["w0a", "w0b", "w1", "w2", "gam", "bet"]:
        w_in[nm] = nc.dram_tensor(nm, [128, 128], f32, kind="ExternalInput")
    for nm in ["b0", "b1", "b2"]:
        w_in[nm] = nc.dram_tensor(nm, [128, 1], f32, kind="ExternalInput")
    out = nc.dram_tensor("out", [BLOCKS_PER_CORE, P, D], f32,
                         kind="ExternalOutput")

    with tile.TileContext(nc) as tc:
        with (
            tc.tile_pool(name="const", bufs=1) as cpool,
            tc.tile_pool(name="edge", bufs=TUNE["ebufs"]) as epool,
            tc.tile_pool(name="oh", bufs=TUNE["ohbufs"]) as ohpool,
            tc.tile_pool(name="small", bufs=TUNE["sbufs"]) as spool,
            tc.tile_pool(name="psag", bufs=TUNE["agbufs"],
                         space="PSUM") as psag,
            tc.tile_pool(name="psmlp", bufs=TUNE["mlpbufs"],
                         space="PSUM") as psmlp,
        ):
            cdma = nc.scalar if TUNE["dma_split"] else nc.sync
            colf_s = cpool.tile([P, tot_c], f16, tag="colf", name="colf")
            cdma.dma_start(out=colf_s[:], in_=colf[:])
            colf32_s = cpool.tile([P, tot_c], f32, tag="colf32",
                                  name="colf32")
            cdma.dma_start(out=colf32_s[:], in_=colf32[:])
            natT_s = cpool.tile([P, NODES_PER_CORE], f32, tag="natT",
                                name="natT")
            cdma.dma_start(out=natT_s[:], in_=natT[:])
            iota_s = cpool.tile([P, kmax, 128], f16, tag="iota", name="iota")
            cdma.dma_start(out=iota_s[:], in_=iota[:])
            consts = {}
            for nm, t in w_in.items():
                consts[nm] = cpool.tile(list(t.shape), f32, tag=nm, name=nm)
                cdma.dma_start(out=consts[nm][:], in_=t[:])
            ident = cpool.tile([P, P], f32, tag="ident", name="ident")
            make_identity(nc, ident[:])
            epst = cpool.tile([P, 1], f32, tag="eps", name="eps")
            nc.vector.memset(epst[:], EPS)

            loop_cm = (tc.For_i(0, loop_iters, 1) if loop_iters
                       else contextlib.nullcontext())
            with loop_cm:
                _emit_blocks(nc, tc, kb, epool, ohpool, spool, psag, psmlp,
                             colf_s, colf32_s, natT_s, iota_s, consts, ident,
                             epst, edges, out, mybir)
    nc.finalize()
    return nc


def _emit_blocks(nc, tc, kb, epool, ohpool, spool, psag, psmlp, colf_s,
                 colf32_s, natT_s, iota_s, consts, ident, epst, edges, out,
                 mybir):
    dt = mybir.dt
    f32 = dt.float32
    f16 = dt.float16
    Alu = mybir.AluOpType
    Act = mybir.ActivationFunctionType
    kmax = max(kb)
    e_off = 0
    c_off = 0
    pair = TUNE["edge_pair"]
    only = TUNE["only"]
    do_dma = only in (None, "dma")
    do_oh = only in (None, "dve")
    do_mm = only in (None, "pe")
    do_mlp = only is None
    pair_tile = None
    pair_off = 0
    eblk0 = None
    oh0 = None
    if only == "pe":
        # static operands loaded once; PE work only
        eblk0 = epool.tile([P, 2 * kmax * 128], f16, tag="eblk", name="eblk")
        nc.sync.dma_start(out=eblk0[:], in_=edges[:, :2 * kmax * 128])
        oh0 = ohpool.tile([P, kmax, 128], f16, tag="oh", name="oh")
        csl0 = colf_s[:, 0:kmax].broadcast_to([P, kmax, 128])
        nc.vector.tensor_tensor(out=oh0[:], in0=csl0, in1=iota_s[:],
                                op=Alu.is_equal)
    for b in range(BLOCKS_PER_CORE):
        K = kb[b]
        KE = K * 128
        edma = (nc.sync if (not TUNE["dma_split"] or b % 2 == 0)
                else nc.scalar)
        odma = (nc.scalar if (not TUNE["dma_split"] or b % 2 == 0)
                else nc.sync) if TUNE["dma_split"] else nc.sync
        if do_dma:
            if pair:
                if b % 2 == 0:
                    hi_b = min(b + 1, BLOCKS_PER_CORE - 1)
                    span = sum(2 * kb[i] * 128 for i in range(b, hi_b + 1))
                    pair_tile = epool.tile([P, 4 * kmax * 128], f16,
                                           tag="eblk", name="eblk")
                    edma.dma_start(out=pair_tile[:, :span],
                                   in_=edges[:, e_off:e_off + span])
                    pair_off = 0
                eblk = pair_tile[:, pair_off:pair_off + 2 * KE]
                pair_off += 2 * KE
            else:
                eblk = epool.tile([P, 2 * kmax * 128], f16, tag="eblk",
                                  name="eblk")
                edma.dma_start(out=eblk[:, :2 * KE],
                               in_=edges[:, e_off:e_off + 2 * KE])
        else:
            eblk = eblk0
        e_off += 2 * KE
        if only == "dma":
            c_off += K
            continue
        if only == "pe":
            pag = psag.tile([P, 128], f32, tag="ag", name="ag")
            for k in range(K):
                nc.tensor.matmul(out=pag[:],
                                 lhsT=eblk[:, k * 128:(k + 1) * 128],
                                 rhs=oh0[:, k, :],
                                 start=(k == 0), stop=False)
                nc.tensor.matmul(out=pag[:],
                                 lhsT=eblk[:, KE + k * 128:KE + (k + 1) * 128],
                                 rhs=oh0[:, k, :],
                                 start=False, stop=(k == K - 1))
            c_off += K
            continue

        # one-hot: oh[p, k, j] = (colf[p, c_off + k] == j)
        oh = ohpool.tile([P, kmax, 128], f16, tag="oh", name="oh")
        if TUNE["oh_mode"] == "tt":
            # whole block in one DVE tensor_tensor (runs at 1x: broadcast in0)
            csl = colf_s[:, c_off:c_off + K].broadcast_to([P, K, 128])
            nc.vector.tensor_tensor(out=oh[:, :K, :], in0=csl,
                                    in1=iota_s[:, :K, :], op=Alu.is_equal)
        else:
            # per-tile tensor_scalar (fp16 single-src: 4x mode)
            for k in range(K):
                nc.vector.tensor_scalar(
                    out=oh[:, k, :], in0=iota_s[:, 0, :],
                    scalar1=colf32_s[:, c_off + k:c_off + k + 1],
                    scalar2=None, op0=Alu.is_equal)
        if only == "dve":
            c_off += K
            continue

        # aggrT[f, j] = sum_k sum_e (hi[e,f] + lo[e,f]) * oh[e, k, j]
        pag = psag.tile([P, 128], f32, tag="ag", name="ag")
        for k in range(K):
            nc.tensor.matmul(out=pag[:],
                             lhsT=eblk[:, k * 128:(k + 1) * 128],
                             rhs=oh[:, k, :],
                             start=(k == 0), stop=False)
            nc.tensor.matmul(out=pag[:],
                             lhsT=eblk[:, KE + k * 128:KE + (k + 1) * 128],
                             rhs=oh[:, k, :],
                             start=False, stop=(k == K - 1))
        aggrT = spool.tile([P, 128], f32, tag="aggrT", name="aggrT")
        nc.scalar.copy(aggrT[:], pag[:])

        # h1T = relu(W0a.T @ natT_blk + W0b.T @ aggrT + b0)
        ph1 = psmlp.tile([P, 128], f32, tag="mlp", name="mlp")
        nc.tensor.matmul(out=ph1[:], lhsT=consts["w0a"][:],
                         rhs=natT_s[:, b * 128:(b + 1) * 128],
                         start=True, stop=False)
        nc.tensor.matmul(out=ph1[:], lhsT=consts["w0b"][:],
                         rhs=aggrT[:], start=False, stop=True)
        h1 = spool.tile([P, 128], f32, tag="h1", name="h1")
        nc.scalar.activation(h1[:], ph1[:], Act.Relu, bias=consts["b0"][:])

        ph2 = psmlp.tile([P, 128], f32, tag="mlp", name="mlp")
        nc.tensor.matmul(out=ph2[:], lhsT=consts["w1"][:], rhs=h1[:],
                         start=True, stop=True)
        h2 = spool.tile([P, 128], f32, tag="h2", name="h2")
        nc.scalar.activation(h2[:], ph2[:], Act.Relu, bias=consts["b1"][:])

        ph3 = psmlp.tile([P, 128], f32, tag="mlp", name="mlp")
        nc.tensor.matmul(out=ph3[:], lhsT=consts["w2"][:], rhs=h2[:],
                         start=True, stop=True)
        h3T = spool.tile([P, 128], f32, tag="h3T", name="h3T")
        nc.scalar.activation(h3T[:], ph3[:], Act.Identity,
                             bias=consts["b2"][:])

        # transpose to node-major, then LayerNorm over features
        py = psmlp.tile([P, 128], f32, tag="mlp", name="mlp")
        nc.tensor.transpose(py[:], h3T[:], ident[:])
        y = spool.tile([P, 128], f32, tag="y", name="y")
        nc.scalar.copy(y[:], py[:])

        stats = spool.tile([P, 6], f32, tag="stats", name="stats")
        nc.vector.bn_stats(stats[:], y[:])
        mv = spool.tile([P, 2], f32, tag="mv", name="mv")
        nc.vector.bn_aggr(mv[:], stats[:])
        std = spool.tile([P, 1], f32, tag="std", name="std")
        nc.scalar.activation(std[:], mv[:, 1:2], Act.Sqrt, bias=epst[:])
        rstd = spool.tile([P, 1], f32, tag="rstd", name="rstd")
        nc.vector.reciprocal(rstd[:], std[:])
        xn = spool.tile([P, 128], f32, tag="xn", name="xn")
        nc.vector.tensor_scalar(out=xn[:], in0=y[:], scalar1=mv[:, 0:1],
                                scalar2=rstd[:], op0=Alu.subtract,
                                op1=Alu.mult)
        g1 = spool.tile([P, 128], f32, tag="g1", name="g1")
        nc.gpsimd.tensor_tensor(out=g1[:], in0=xn[:], in1=consts["gam"][:],
                                op=Alu.mult)
        yo = spool.tile([P, 128], f32, tag="yo", name="yo")
        nc.gpsimd.tensor_tensor(out=yo[:], in0=g1[:], in1=consts["bet"][:],
                                op=Alu.add)
        odma.dma_start(out=out[b], in_=yo[:])
        c_off += K


def _prepare_shards(node_attr, edge_attr, col):
    """Bucket edges by destination-node block; build per-core arrays."""
    E = col.shape[0]
    blk = col >> 7                                  # global block id
    counts = np.bincount(blk, minlength=TOTAL_BLOCKS)
    kb = np.ceil(np.maximum(
        counts.reshape(N_CORES, BLOCKS_PER_CORE).max(axis=0), 1) / 128
    ).astype(np.int64)                              # [100] per-position tiles
    kbe = kb * 128
    blk_start = np.zeros(BLOCKS_PER_CORE + 1, np.int64)
    blk_start[1:] = np.cumsum(kbe)                  # slot base per position
    slots_per_core = int(blk_start[-1])

    order = np.argsort(blk, kind="stable")
    starts = np.zeros(TOTAL_BLOCKS + 1, np.int64)
    starts[1:] = np.cumsum(counts)
    blk_sorted = blk[order]
    within = np.arange(E, dtype=np.int64) - starts[blk_sorted]
    col_local_sorted = (col[order] & 127).astype(np.float16)

    # edges layout per core: [P, sum_b 2*kbe[b]] fp16 (hi block then lo block)
    edges_by_core = []
    colf_by_core = []
    natp = np.zeros((N_CORES * NODES_PER_CORE, D), np.float32)
    natp[:NUM_NODES] = node_attr
    natT_by_core = []
    for c in range(N_CORES):
        lo_i = int(starts[c * BLOCKS_PER_CORE])
        hi_i = int(starts[(c + 1) * BLOCKS_PER_CORE])
        loc_blk = blk_sorted[lo_i:hi_i] - c * BLOCKS_PER_CORE
        slot = blk_start[loc_blk] + within[lo_i:hi_i]
        ebuf = np.zeros((slots_per_core, D), np.float32)
        ebuf[slot] = edge_attr[order[lo_i:hi_i]]
        ehi = ebuf.astype(np.float16)
        elo = (ebuf - ehi.astype(np.float32)).astype(np.float16)
        cbuf = np.full((slots_per_core,), -1.0, np.float16)
        cbuf[slot] = col_local_sorted[lo_i:hi_i]

        earr = np.empty((P, int(2 * kbe.sum())), np.float16)
        carr = np.empty((P, int(kb.sum())), np.float16)
        e_off = 0
        c_off = 0
        for b in range(BLOCKS_PER_CORE):
            s0, s1 = int(blk_start[b]), int(blk_start[b + 1])
            K = int(kb[b])
            KE = K * 128
            earr[:, e_off:e_off + KE] = (
                ehi[s0:s1].reshape(K, 128, D).transpose(1, 0, 2)
                .reshape(P, K * 128))
            earr[:, e_off + KE:e_off + 2 * KE] = (
                elo[s0:s1].reshape(K, 128, D).transpose(1, 0, 2)
                .reshape(P, K * 128))
            carr[:, c_off:c_off + K] = cbuf[s0:s1].reshape(K, 128).T
            e_off += 2 * KE
            c_off += K
        edges_by_core.append(earr)
        colf_by_core.append(carr)
        natT_by_core.append(np.ascontiguousarray(
            natp[c * NODES_PER_CORE:(c + 1) * NODES_PER_CORE].T))
    return tuple(int(x) for x in kb), edges_by_core, colf_by_core, \
        natT_by_core


def kernel(node_attr, edge_attr, edge_index, W0, b0, W1, b1, W2, b2,
           ln_g, ln_b):
    from concourse import bass_utils

    node_attr = np.ascontiguousarray(np.asarray(node_attr, dtype=np.float32))
    edge_attr = np.ascontiguousarray(np.asarray(edge_attr, dtype=np.float32))
    col = np.asarray(edge_index)[1].astype(np.int64)
    W0 = np.asarray(W0, dtype=np.float32)
    W1 = np.ascontiguousarray(np.asarray(W1, dtype=np.float32))
    W2 = np.ascontiguousarray(np.asarray(W2, dtype=np.float32))
    b0v = np.asarray(b0, dtype=np.float32).reshape(128, 1).copy()
    b1v = np.asarray(b1, dtype=np.float32).reshape(128, 1).copy()
    b2v = np.asarray(b2, dtype=np.float32).reshape(128, 1).copy()
    gam = np.ascontiguousarray(
        np.tile(np.asarray(ln_g, np.float32).reshape(1, 128), (128, 1)))
    bet = np.ascontiguousarray(
        np.tile(np.asarray(ln_b, np.float32).reshape(1, 128), (128, 1)))

    kb, edges_by_core, colf_by_core, natT_by_core = _prepare_shards(
        node_attr, edge_attr, col)
    kmax = max(kb)

    iota_rep = np.ascontiguousarray(
        np.broadcast_to(np.arange(128, dtype=np.float16), (P, kmax, 128)))
    w0a = np.ascontiguousarray(W0[:128])
    w0b = np.ascontiguousarray(W0[128:])

    if kb not in _nc_cache:
        _nc_cache[kb] = _build_nc(kb)
    nc = _nc_cache[kb]

    shared = {"iota": iota_rep, "w0a": w0a, "w0b": w0b, "w1": W1, "w2": W2,
              "b0": b0v, "b1": b1v, "b2": b2v, "gam": gam, "bet": bet}
    in_maps = []
    for c in range(N_CORES):
        m = {"edges": edges_by_core[c], "colf": colf_by_core[c],
             "colf32": colf_by_core[c].astype(np.float32),
             "natT": natT_by_core[c]}
        m.update(shared)
        in_maps.append(m)

    res = bass_utils.run_bass_kernel_spmd(nc, in_maps,
                                          core_ids=list(range(N_CORES)))
    last_run_info["results"] = res
    last_run_info["nc"] = nc
    last_run_info["in_maps"] = in_maps
    last_run_info["kb"] = kb

    outs = [res.results[c]["out"].reshape(NODES_PER_CORE, D)
            for c in range(N_CORES)]
    return np.concatenate(outs, axis=0)[:NUM_NODES].astype(np.float32)



# revision 10
# speedup vs baseline: 9.0851x; 9.0851x over previous
"""Trainium2 Bass kernel for nn_NodeBlock (GNN message passing).

Pipeline: segment_sum of edge features onto destination nodes, concat with
node features, 3-layer MLP, LayerNorm.

Sharding: nodes are range-sharded across the 8 cores (12800 nodes/core, 100
blocks of 128). On the host, edges are bucketed by destination-node block.
Within a block, the first T edges of each node are packed "dense":
feature-major tiles dT[f, t*128 + j] = (t-th edge of node j)[f], so the
on-device aggregation for them is a plain PSUM accumulation via an
identity-weight matmul (no one-hot needed). Edges beyond T per node go to
"overflow" tiles in slot-major layout with a per-tile one-hot built by a
DVE is_equal against an iota row (like a classic scatter-add matmul).

All edge/node/weight data is fp16 (rel err ~3e-4, far inside the 2e-2
budget), so the PE runs at 1 cycle/row and edge HBM traffic is 2B/elem.

MLP runs feature-major, batched over groups of 4 blocks (free dim 512).
LayerNorm: W2 is column-centered on the host so the mean term vanishes;
variance comes from an ACT Square+accum_out on the transposed block;
rstd is folded into the PSUM evacuation; gamma/beta are folded into one
ACT (scale/bias per-partition) after transposing back to feature-major.
Output is fp16 feature-major; the host transposes/upcasts.
"""

import sys

sys.path.insert(0, "/opt/trn_rl_repo")

import numpy as np

N_CORES = 8
NUM_NODES = 100000
D = 128            # node/edge feature dim
P = 128            # partitions
BLK = 128          # nodes per block
BLOCKS_PER_CORE = 100
NODES_PER_CORE = BLK * BLOCKS_PER_CORE   # 12800
TOTAL_BLOCKS = N_CORES * BLOCKS_PER_CORE  # 800
GRP = 4            # blocks per MLP group (free dim 512)
NGRP = BLOCKS_PER_CORE // GRP            # 25
EPS = 1e-5

_nc_cache = {}
last_run_info = {}

TUNE = {
    "T": 14,            # dense depth (edges per node packed densely)
    "chunk": 2,         # blocks per edge DMA
    "ebufs": 4, "ohbufs": 4, "sbufs": 3,
    "agbufs": 2, "mlpbufs": 2, "tybufs": 2, "txbufs": 2,
    "oh_engine": "vector",      # vector | gpsimd | split
    "aggr_evac": "vector",      # vector | scalar
    "relu_engine": "scalar",    # scalar | vector
    "stats_engine": "scalar",   # scalar (Square+accum) | vector (bn_stats)
    "yot_engine": "scalar",     # scalar | gpsimd
    "xn_engine": "vector",      # vector | scalar
}


def _build_nc(kb, loop_iters=None):
    """kb: ("v2", T, (V_b per block position,) * 100)."""
    import contextlib
    import concourse.bacc as bacc
    import concourse.tile as tile
    import concourse.mybir as mybir
    from concourse.masks import make_identity

    dt = mybir.dt
    f32 = dt.float32
    f16 = dt.float16
    _, T, vb = kb
    vb = list(vb)
    tot_v = sum(vb)                      # total overflow tiles per core
    tot_e = sum((T + v) * 128 for v in vb)   # per-partition fp16 elems

    nc = bacc.Bacc("TRN2", target_bir_lowering=False, debug=False,
                   name="nodeblock")

    edges = nc.dram_tensor("edges", [P, tot_e], f16, kind="ExternalInput")
    iota_in = nc.dram_tensor("iota", [P, P], f16, kind="ExternalInput")
    colv = nc.dram_tensor("colv", [P, max(tot_v, 1)], f32,
                          kind="ExternalInput")
    natT = nc.dram_tensor("natT", [P, NODES_PER_CORE], f16,
                          kind="ExternalInput")
    w_in = {}
    for nm in ["w0a", "w0b", "w1", "w2c"]:
        w_in[nm] = nc.dram_tensor(nm, [128, 128], f16, kind="ExternalInput")
    for nm in ["b0", "b1", "b2c", "gam", "bet"]:
        w_in[nm] = nc.dram_tensor(nm, [128, 1], f32, kind="ExternalInput")
    out = nc.dram_tensor("out", [NGRP, P, GRP * BLK], f16,
                         kind="ExternalOutput")

    with tile.TileContext(nc) as tc:
        with (
            tc.tile_pool(name="const", bufs=1) as cpool,
            tc.tile_pool(name="edge", bufs=TUNE["ebufs"]) as epool,
            tc.tile_pool(name="oh", bufs=TUNE["ohbufs"]) as ohpool,
            tc.tile_pool(name="small", bufs=TUNE["sbufs"]) as spool,
            tc.tile_pool(name="psag", bufs=TUNE["agbufs"],
                         space="PSUM") as psag,
            tc.tile_pool(name="psmlp", bufs=TUNE["mlpbufs"],
                         space="PSUM") as psmlp,
            tc.tile_pool(name="psty", bufs=TUNE["tybufs"],
                         space="PSUM") as psty,
            tc.tile_pool(name="pstx", bufs=TUNE["txbufs"],
                         space="PSUM") as pstx,
        ):
            colv_s = cpool.tile([P, max(tot_v, 1)], f32, tag="colv",
                                name="colv")
            nc.scalar.dma_start(out=colv_s[:], in_=colv[:])
            natT_s = cpool.tile([P, NODES_PER_CORE], f16, tag="natT",
                                name="natT")
            nc.scalar.dma_start(out=natT_s[:], in_=natT[:])
            consts = {}
            for nm, t in w_in.items():
                cdt = f16 if nm in ("w0a", "w0b", "w1", "w2c") else f32
                consts[nm] = cpool.tile(list(t.shape), cdt, tag=nm, name=nm)
                nc.scalar.dma_start(out=consts[nm][:], in_=t[:])
            ident = cpool.tile([P, P], f16, tag="ident", name="ident")
            make_identity(nc, ident[:])
            iota = cpool.tile([P, P], f16, tag="iota", name="iota")
            nc.scalar.dma_start(out=iota[:], in_=iota_in[:])
            consts["iota"] = iota
            epst = cpool.tile([P, 1], f32, tag="eps", name="eps")
            nc.vector.memset(epst[:], EPS)
            consts["eps"] = epst

            loop_cm = (tc.For_i(0, loop_iters, 1) if loop_iters
                       else contextlib.nullcontext())
            with loop_cm:
                _emit_blocks(nc, tc, T, vb, epool, ohpool, spool, psag,
                             psmlp, psty, pstx, colv_s, natT_s, consts,
                             ident, edges, out, mybir)
    nc.finalize()
    return nc


def _emit_blocks(nc, tc, T, vb, epool, ohpool, spool, psag, psmlp, psty,
                 pstx, colv_s, natT_s, consts, ident, edges, out, mybir):
    dt = mybir.dt
    f32 = dt.float32
    f16 = dt.float16
    Alu = mybir.AluOpType
    Act = mybir.ActivationFunctionType
    chunk = TUNE["chunk"]
    iota = consts["iota"]

    # max chunk span in per-partition elems (for tile sizing)
    spans = []
    for b0 in range(0, BLOCKS_PER_CORE, chunk):
        spans.append(sum((T + vb[b]) * 128
                         for b in range(b0, min(b0 + chunk,
                                                BLOCKS_PER_CORE))))
    max_span = max(spans)

    e_off = 0
    v_off = 0
    chunk_tile = None
    chunk_off = 0
    for g in range(NGRP):
        pag = psag.tile([P, GRP * BLK], f32, tag="ag", name="ag")
        ssq = spool.tile([P, GRP], f32, tag="ssq", name="ssq")
        py_tiles = []
        for q in range(GRP):
            b = g * GRP + q
            V = vb[b]
            span = (T + V) * 128
            if b % chunk == 0:
                hi = min(b + chunk, BLOCKS_PER_CORE)
                cspan = sum((T + vb[i]) * 128 for i in range(b, hi))
                chunk_tile = epool.tile([P, max_span], f16, tag="eblk",
                                        name="eblk")
                edma = nc.sync if (b // chunk) % 2 == 0 else nc.scalar
                edma.dma_start(out=chunk_tile[:, :cspan],
                               in_=edges[:, e_off:e_off + cspan])
                chunk_off = 0
            eblk = chunk_tile[:, chunk_off:chunk_off + span]
            chunk_off += span
            e_off += span

            pq = pag[:, q * BLK:(q + 1) * BLK]
            # dense tiles: feature-major, identity weights -> pure accum
            for t in range(T):
                nc.tensor.matmul(out=pq, lhsT=ident[:],
                                 rhs=eblk[:, t * 128:(t + 1) * 128],
                                 start=(t == 0), stop=(t == T - 1 and V == 0),
                                 skip_group_check=True)
            # overflow tiles: slot-major with one-hot rhs
            for v in range(V):
                oh = ohpool.tile([P, 128], f16, tag="oh", name="oh")
                oeng = nc.vector
                if TUNE["oh_engine"] == "gpsimd":
                    oeng = nc.gpsimd
                elif TUNE["oh_engine"] == "split" and v % 2 == 1:
                    oeng = nc.gpsimd
                oeng.tensor_scalar(
                    out=oh[:], in0=iota[:],
                    scalar1=colv_s[:, v_off + v:v_off + v + 1],
                    scalar2=None, op0=Alu.is_equal)
                nc.tensor.matmul(
                    out=pq,
                    lhsT=eblk[:, (T + v) * 128:(T + v + 1) * 128],
                    rhs=oh[:], start=False, stop=(v == V - 1),
                    skip_group_check=True)
            v_off += V

        # aggregated fp16 copy PSUM -> SBUF
        aggrT = spool.tile([P, GRP * BLK], f16, tag="aggrT", name="aggrT")
        if TUNE["aggr_evac"] == "vector":
            nc.vector.tensor_copy(aggrT[:], pag[:])
        else:
            nc.scalar.copy(aggrT[:], pag[:])

        # MLP over the whole group (free dim 512)
        ph1 = psmlp.tile([P, GRP * BLK], f32, tag="mlp", name="mlp")
        nc.tensor.matmul(out=ph1[:], lhsT=consts["w0a"][:],
                         rhs=natT_s[:, g * GRP * BLK:(g + 1) * GRP * BLK],
                         start=True, stop=False)
        nc.tensor.matmul(out=ph1[:], lhsT=consts["w0b"][:],
                         rhs=aggrT[:], start=False, stop=True)
        h1 = spool.tile([P, GRP * BLK], f16, tag="h1", name="h1")
        if TUNE["relu_engine"] == "scalar":
            nc.scalar.activation(h1[:], ph1[:], Act.Relu,
                                 bias=consts["b0"][:])
        else:
            nc.vector.tensor_scalar(out=h1[:], in0=ph1[:],
                                    scalar1=consts["b0"][:], scalar2=0.0,
                                    op0=Alu.add, op1=Alu.max)

        ph2 = psmlp.tile([P, GRP * BLK], f32, tag="mlp", name="mlp")
        nc.tensor.matmul(out=ph2[:], lhsT=consts["w1"][:], rhs=h1[:],
                         start=True, stop=True)
        h2 = spool.tile([P, GRP * BLK], f16, tag="h2", name="h2")
        nc.scalar.activation(h2[:], ph2[:], Act.Relu, bias=consts["b1"][:])

        ph3 = psmlp.tile([P, GRP * BLK], f32, tag="mlp", name="mlp")
        nc.tensor.matmul(out=ph3[:], lhsT=consts["w2c"][:], rhs=h2[:],
                         start=True, stop=True)
        h3T = spool.tile([P, GRP * BLK], f16, tag="h3T", name="h3T")
        nc.scalar.activation(h3T[:], ph3[:], Act.Identity,
                             bias=consts["b2c"][:])

        # per block: transpose to node-major, Square+accum for variance
        py_g = psty.tile([P, GRP * BLK], f16, tag="py", name="py")
        for q in range(GRP):
            py = py_g[:, q * BLK:(q + 1) * BLK]
            nc.tensor.transpose(py, h3T[:, q * BLK:(q + 1) * BLK],
                                ident[:])
            py_tiles.append(py)
            if TUNE["stats_engine"] == "scalar":
                sq = spool.tile([P, BLK], f16, tag="sq", name="sq")
                nc.scalar.activation(sq[:], py, Act.Square,
                                     accum_out=ssq[:, q:q + 1])
            else:
                st6 = spool.tile([P, 6], f32, tag="st6", name="st6")
                nc.vector.bn_stats(st6[:], py)
                nc.vector.bn_aggr(ssq[:, q:q + 1], st6[:])

        # rstd for the 4 blocks at once: 1/sqrt(ssq/128 + eps)
        std = spool.tile([P, GRP], f32, tag="std", name="std")
        nc.scalar.activation(std[:], ssq[:], Act.Sqrt,
                             bias=consts["eps"][:], scale=1.0 / BLK)
        rstd = spool.tile([P, GRP], f32, tag="rstd", name="rstd")
        nc.vector.reciprocal(rstd[:], std[:])

        # xn = py * rstd (node-major), transpose back, fold gamma/beta
        pxt = pstx.tile([P, GRP * BLK], f16, tag="pxt", name="pxt")
        for q in range(GRP):
            xn = spool.tile([P, BLK], f16, tag="xn", name="xn")
            if TUNE["xn_engine"] == "vector":
                nc.vector.tensor_scalar(out=xn[:], in0=py_tiles[q],
                                        scalar1=rstd[:, q:q + 1],
                                        scalar2=None, op0=Alu.mult)
            else:
                nc.scalar.activation(xn[:], py_tiles[q], Act.Copy,
                                     scale=rstd[:, q:q + 1])
            nc.tensor.transpose(pxt[:, q * BLK:(q + 1) * BLK], xn[:],
                                ident[:])
        yoT = spool.tile([P, GRP * BLK], f16, tag="yoT", name="yoT")
        if TUNE["yot_engine"] == "scalar":
            nc.scalar.activation(yoT[:], pxt[:], Act.Identity,
                                 bias=consts["bet"][:],
                                 scale=consts["gam"][:])
        else:
            nc.gpsimd.tensor_scalar(out=yoT[:], in0=pxt[:],
                                    scalar1=consts["gam"][:],
                                    scalar2=consts["bet"][:],
                                    op0=Alu.mult, op1=Alu.add)
        odma = nc.scalar if g % 2 == 0 else nc.sync
        odma.dma_start(out=out[g], in_=yoT[:])


def _prepare_shards(node_attr, edge_attr, col, T):
    """Bucket edges: dense (first T per node, feature-major) + overflow."""
    E = col.shape[0]
    deg = np.bincount(col, minlength=N_CORES * NODES_PER_CORE)

    order = np.argsort(col, kind="stable")
    col_s = col[order]
    starts = np.zeros(N_CORES * NODES_PER_CORE + 1, np.int64)
    starts[1:] = np.cumsum(deg)
    rank = np.arange(E, dtype=np.int64) - starts[col_s]

    # ---- dense part: [node, t, feat] then per-core feature-major tiles
    dense = np.zeros((N_CORES * NODES_PER_CORE, T, D), np.float16)
    mask_d = rank < T
    dense[col_s[mask_d], rank[mask_d]] = \
        edge_attr[order[mask_d]].astype(np.float16)

    # ---- overflow: per (core, pos) sorted runs
    mask_o = ~mask_d
    ocol = col_s[mask_o]                       # sorted by node
    oval = edge_attr[order[mask_o]].astype(np.float16)
    oblk = ocol >> 7                           # global block id
    ocnt = np.bincount(oblk, minlength=TOTAL_BLOCKS)
    vb_cb = np.ceil(ocnt / 128).astype(np.int64).reshape(
        N_CORES, BLOCKS_PER_CORE)
    vb = vb_cb.max(axis=0)                     # [100] shared across cores
    vb = np.maximum(vb, 0)

    ostarts = np.zeros(TOTAL_BLOCKS + 1, np.int64)
    ostarts[1:] = np.cumsum(ocnt)
    o_within = np.arange(len(ocol), dtype=np.int64) - ostarts[oblk]

    # per-block per-partition elems and offsets
    blk_elems = (T + vb) * 128                 # [100]
    blk_off = np.zeros(BLOCKS_PER_CORE + 1, np.int64)
    blk_off[1:] = np.cumsum(blk_elems)
    tot_e = int(blk_off[-1])
    tot_v = int(vb.sum())
    v_off = np.zeros(BLOCKS_PER_CORE + 1, np.int64)
    v_off[1:] = np.cumsum(vb)

    natp = np.zeros((N_CORES * NODES_PER_CORE, D), np.float32)
    natp[:NUM_NODES] = node_attr

    edges_by_core = []
    colv_by_core = []
    natT_by_core = []
    for c in range(N_CORES):
        earr = np.zeros((P, tot_e), np.float16)
        carr = np.full((P, max(tot_v, 1)), -1.0, np.float32)

        # dense: [pos, j, t, f] -> [f, pos, t*128 + j]
        dc = dense[c * NODES_PER_CORE:(c + 1) * NODES_PER_CORE]
        dc = dc.reshape(BLOCKS_PER_CORE, BLK, T, D)
        dcT = np.ascontiguousarray(dc.transpose(3, 0, 2, 1))  # [f,pos,t,j]
        for b in range(BLOCKS_PER_CORE):
            earr[:, blk_off[b]:blk_off[b] + T * 128] = \
                dcT[:, b].reshape(P, T * 128)

        # overflow for this core
        lo = int(ostarts[c * BLOCKS_PER_CORE])
        hi = int(ostarts[(c + 1) * BLOCKS_PER_CORE])
        if hi > lo:
            loc_blk = oblk[lo:hi] - c * BLOCKS_PER_CORE
            w = o_within[lo:hi]
            vt = w >> 7                        # overflow tile within block
            sl = w & 127                       # slot within tile
            # edge values: earr[sl(partition), blk_off + (T+vt)*128 + f]
            base = blk_off[loc_blk] + (T + vt) * 128
            earr[sl[:, None],
                 (base[:, None] + np.arange(D)[None, :])] = oval[lo:hi]
            carr[sl, v_off[loc_blk] + vt] = (ocol[lo:hi] & 127).astype(
                np.float32)

        edges_by_core.append(earr)
        colv_by_core.append(carr)
        natT_by_core.append(np.ascontiguousarray(
            natp[c * NODES_PER_CORE:(c + 1) * NODES_PER_CORE].T.astype(
                np.float16)))
    return tuple(int(x) for x in vb), edges_by_core, colv_by_core, \
        natT_by_core


def assemble_core_out(arr):
    """[NGRP, 128, GRP*128] fp16 feature-major -> [12800, 128] f32."""
    a = np.asarray(arr).reshape(NGRP, P, GRP, BLK)
    return a.transpose(0, 2, 3, 1).reshape(NODES_PER_CORE, D).astype(
        np.float32)


def kernel(node_attr, edge_attr, edge_index, W0, b0, W1, b1, W2, b2,
           ln_g, ln_b):
    from concourse import bass_utils

    T = TUNE["T"]
    node_attr = np.ascontiguousarray(np.asarray(node_attr, dtype=np.float32))
    edge_attr = np.ascontiguousarray(np.asarray(edge_attr, dtype=np.float32))
    col = np.asarray(edge_index)[1].astype(np.int64)
    W0 = np.asarray(W0, dtype=np.float64)
    W1 = np.asarray(W1, dtype=np.float64)
    W2 = np.asarray(W2, dtype=np.float64)
    b2v = np.asarray(b2, dtype=np.float64)
    # center W2 columns (per output feature) so LN mean vanishes
    W2c = W2 - W2.mean(axis=1, keepdims=True)
    b2c = b2v - b2v.mean()

    w0a = np.ascontiguousarray(W0[:128].astype(np.float16))
    w0b = np.ascontiguousarray(W0[128:].astype(np.float16))
    w1 = np.ascontiguousarray(W1.astype(np.float16))
    w2c = np.ascontiguousarray(W2c.astype(np.float16))
    b0v = np.asarray(b0, np.float32).reshape(128, 1).copy()
    b1v = np.asarray(b1, np.float32).reshape(128, 1).copy()
    b2cv = b2c.astype(np.float32).reshape(128, 1).copy()
    gam = np.asarray(ln_g, np.float32).reshape(128, 1).copy()
    bet = np.asarray(ln_b, np.float32).reshape(128, 1).copy()

    vb, edges_by_core, colv_by_core, natT_by_core = _prepare_shards(
        node_attr, edge_attr, col, T)

    kb = ("v2", T, vb)
    if kb not in _nc_cache:
        _nc_cache[kb] = _build_nc(kb)
    nc = _nc_cache[kb]

    iota_rep = np.ascontiguousarray(
        np.broadcast_to(np.arange(128, dtype=np.float16), (P, P)))
    shared = {"w0a": w0a, "w0b": w0b, "w1": w1, "w2c": w2c,
              "b0": b0v, "b1": b1v, "b2c": b2cv, "gam": gam, "bet": bet,
              "iota": iota_rep}
    in_maps = []
    for c in range(N_CORES):
        m = {"edges": edges_by_core[c], "colv": colv_by_core[c],
             "natT": natT_by_core[c]}
        m.update(shared)
        in_maps.append(m)

    res = bass_utils.run_bass_kernel_spmd(nc, in_maps,
                                          core_ids=list(range(N_CORES)))
    last_run_info["results"] = res
    last_run_info["nc"] = nc
    last_run_info["in_maps"] = in_maps
    last_run_info["kb"] = kb

    outs = [assemble_core_out(res.results[c]["out"])
            for c in range(N_CORES)]
    return np.concatenate(outs, axis=0)[:NUM_NODES]
